# revision 14
# baseline (speedup 1.0000x reference)
"""GAT layer (nn_GAT_layer_67619965108552) as a Trainium2 Bass/Tile SPMD kernel.

Structure exploited (same math as the verified baseline):
  With n=8192, the buggy-but-faithful pair indexing collapses:
    rows i < 4096:  scores[i, j] = u[2i + (j >= 4096)],  u = x @ (W@a1 + W@a2)
    rows i >= 4096: scores[i, j] = tt[j mod 4096],       tt = s1[even] + s2[odd]
  After leaky_relu + adj masking + softmax, attn @ out reduces to two masked
  row-sum matmuls against adj halves:
    Y1 = A[:, :4096] @ [f*out_L | f],  Y2 = A[:, 4096:] @ [f*out_R | f]
    res = sigmoid((al1*Y1 + al2*Y2)[:, :256] / (al1*Y1 + al2*Y2)[:, 256])
  Top-half cores: f = 1, al1 = exp(lrelu(u_even)), al2 = exp(lrelu(u_odd));
  bottom-half cores: f = exp(lrelu(tt)), al1 = al2 = 1. Same instruction
  stream on all cores; only input data (g / select masks) differs.

Layout strategy (this is where the speed comes from vs the old version):
  * x arrives host-pre-transposed as bf16 xT [512, 8192] -> matmul lhsT/rhs
    tiles load straight from DRAM; zero on-chip transposes.
  * each core's adj slice arrives host-pre-transposed as int8 adjT [8192,1024]
    -> SWDGE dma casts i8->bf16 on load; tiles are directly the lhsT of the
    Y matmuls; zero on-chip transposes and 4x less HBM traffic.
  * U-pass (scores) runs before the out-pass; score vectors scatter to a DRAM
    scratch DURING the U-pass and reload once partition-major, so leaky_relu/
    exp run on [128, 96] tiles and the tensor engine never idles waiting.
  * Stage B accumulates Y in 8 PSUM banks per half directly.

Sharding: rows of adj (and of the output) across 8 cores, 1024 rows each.
x/weight/att_vec replicated; every core computes the full out = x@W.
"""
import numpy as np
from contextlib import ExitStack

import concourse.bass as bass
import concourse.tile as tile
from concourse import bacc, mybir
from concourse.bass_utils import run_bass_kernel_spmd

F32 = mybir.dt.float32
BF16 = mybir.dt.bfloat16
I8 = mybir.dt.int8

N = 8192          # nodes
FIN = 512         # input features
FOUT = 256        # output features
P = 128
NB = N // P       # 64 j-chunks over all nodes
NCORES = 8
RPC = N // NCORES  # 1024 rows per core
MB = RPC // P      # 8 output row-blocks per core
HKC = 32           # j-chunks per half (4096/128)
GJ = 8             # j-chunks per adj DMA group
NG = NB // GJ      # 8 adj groups
UG = 16            # U-pass groups of 512 nodes


def build_program():
    nc = bacc.Bacc("TRN2", target_bir_lowering=False, debug=False,
                   num_devices=NCORES)

    xt_d = nc.dram_tensor("xt", [FIN, N], BF16, kind="ExternalInput")
    w_d = nc.dram_tensor("w", [FIN, FOUT], F32, kind="ExternalInput")
    attb_d = nc.dram_tensor("attb", [P, 2 * FOUT], F32, kind="ExternalInput")
    adjt_d = nc.dram_tensor("adjt", [N, RPC], I8, kind="ExternalInput")
    # gcol[:, 0] = g (1.0 for top-half cores, 0.0 for bottom), gcol[:, 1] = 1-g
    g_d = nc.dram_tensor("gcol", [P, 2], F32, kind="ExternalInput")
    # selg[p, B', B] = g * (B == 8c + B') : per-core row-block select
    selg_d = nc.dram_tensor("selg", [P, MB, HKC], F32, kind="ExternalInput")
    # basis vectors for psum row extraction: col 0 -> row 1, col 1 -> row 2
    eb_d = nc.dram_tensor("ebasis", [3, 2], F32, kind="ExternalInput")
    y_d = nc.dram_tensor("y", [RPC, FOUT], F32, kind="ExternalOutput")

    with tile.TileContext(nc) as tc, ExitStack() as ctx:
        constp = ctx.enter_context(tc.tile_pool(name="const", bufs=1))
        dramp = ctx.enter_context(tc.tile_pool(name="dram", bufs=1, space="DRAM"))
        # adj tiles stream through 3 slots; DMAs issued up-front so they run
        # under stage A (gpsimd/SWDGE queue carries nothing else).
        adjp = ctx.enter_context(tc.tile_pool(name="adjp", bufs=3))

        # ---- constants ----
        wtile = constp.tile([P, 4, FOUT], F32)     # W, k-chunk major
        nc.sync.dma_start(wtile[:], w_d.ap().rearrange("(c p) f -> p c f", p=P))
        wbf = constp.tile([P, 4, FOUT], BF16)
        nc.vector.tensor_copy(wbf[:], wtile[:])
        attb = constp.tile([P, 2 * FOUT], F32)     # [a1 | a2], partition-bcast
        nc.sync.dma_start(attb[:], attb_d.ap())
        gcol = constp.tile([P, 2], F32)
        nc.sync.dma_start(gcol[:], g_d.ap())
        selg = constp.tile([P, MB, HKC], F32)
        nc.sync.dma_start(selg[:], selg_d.ap())
        ebasis = constp.tile([3, 2], F32)
        nc.sync.dma_start(ebasis[:], eb_d.ap())

        # persistent mid-size tensors
        outb = [constp.tile([P, HKC, FOUT + 1], BF16, name=f"outb{h}")
                for h in range(2)]
        zsb = constp.tile([P, MB, FOUT + 1], F32)
        rawpm = constp.tile([P, 3 * HKC], F32)   # [ae_raw | be_raw | tt_raw]
        expv = constp.tile([P, 3 * HKC], F32)    # exp(lrelu(rawpm))
        fpm = constp.tile([P, HKC], F32)
        al1 = constp.tile([P, MB], F32)
        al2 = constp.tile([P, MB], F32)
        wamf = constp.tile([P, 4, 3], F32)
        wam = constp.tile([P, 4, 3], BF16)

        # DRAM scratch for the free-major -> partition-major shuffle of the
        # score vectors: rows = [ae | be | tt], each 4096 long, index-major.
        vecd = dramp.tile([3, N // 2], F32)

        def adj_load(g):
            t = adjp.tile([P, GJ, RPC], BF16, tag="adjg", name=f"adjg{g}")
            nc.gpsimd.dma_start(
                t[:],
                adjt_d.ap()[g * GJ * P:(g + 1) * GJ * P, :].rearrange(
                    "(t p) i -> p t i", p=P))
            return t

        # adj loads are all deferred until after the stage-A gather DMAs:
        # they'd compete with the xT stream for HBM (the U-pass is paced by
        # it) and anything slot-waiting on stage-B consumption must not sit
        # ahead of the gathers in the gpsimd FIFO.
        adjg = []

        # ---- stage A (scoped pools) ----
        with tc.tile_pool(name="xtp", bufs=1) as xtp, \
             tc.tile_pool(name="sa", bufs=3) as sa, \
             tc.tile_pool(name="ps_u", bufs=2, space="PSUM") as ps_u, \
             tc.tile_pool(name="ps_e", bufs=2, space="PSUM") as ps_e, \
             tc.tile_pool(name="ps_a", bufs=4, space="PSUM") as ps_a:

            # resident xT [k-part, c, n] as 8 slice tiles of 1 MB each, so
            # the U-pass only waits on the slices it has reached
            NS = N // 8
            xts = []
            for s in range(8):
                sl = slice(s * NS, (s + 1) * NS)
                xs = xtp.tile([P, 4, NS], BF16, name=f"xt{s}")
                nc.sync.dma_start(
                    xs[:],
                    xt_d.ap()[:, sl].rearrange("(c p) n -> p c n", p=P))
                xts.append(xs)

            def xtsl(n0, n1):
                """[P, 4, n1-n0] view of the xT slice tile covering n0:n1."""
                s = n0 // NS
                assert (n1 - 1) // NS == s
                return xts[s][:, :, n0 - s * NS:n1 - s * NS]

            # wam[:, c, :] = [wu | wa1 | wa2] chunk c (bf16 lhsT for U-matmul)
            for c in range(4):
                t = sa.tile([P, FOUT], F32, tag="wa_tmp", name="wa_tmp")
                nc.vector.tensor_mul(t[:], wtile[:, c, :], attb[:, :FOUT])
                nc.vector.tensor_reduce(wamf[:, c, 1:2], t[:],
                                        axis=mybir.AxisListType.X,
                                        op=mybir.AluOpType.add)
                t2 = sa.tile([P, FOUT], F32, tag="wa_tmp", name="wa_tmp2")
                nc.vector.tensor_mul(t2[:], wtile[:, c, :], attb[:, FOUT:])
                nc.vector.tensor_reduce(wamf[:, c, 2:3], t2[:],
                                        axis=mybir.AxisListType.X,
                                        op=mybir.AluOpType.add)
                nc.vector.tensor_add(wamf[:, c, 0:1], wamf[:, c, 1:2],
                                     wamf[:, c, 2:3])
            nc.vector.tensor_copy(wam[:], wamf[:])

            # ---- U pass interleaved with the out pass, per xT slice ----
            # The xT stream paces stage A; per slice the 2 U-groups run
            # first (the score pipeline is the long dependency chain), then
            # the 8 out-blocks of the same slice fill the tensor engine
            # until the next slice lands.
            def u_group(g):
                pu = ps_u.tile([3, 512], F32, tag="pu", name="pu")
                for c in range(4):
                    nc.tensor.matmul(pu[:], wam[:, c, :],
                                     xtsl(g * 512, (g + 1) * 512)[:, c, :],
                                     start=(c == 0), stop=(c == 3))
                pusb = sa.tile([3, 512], F32, tag="pusb", name="pusb")
                if g % 2 == 0:
                    nc.vector.tensor_copy(pusb[:], pu[:])
                else:
                    nc.scalar.copy(pusb[:], pu[:])
                # tt = s1[even] + s2[odd]: two basis extractions into one PSUM
                ext = ps_e.tile([1, 256], F32, tag="ext", name="ext")
                nc.tensor.matmul(ext[:], ebasis[:, 0:1], pusb[:, 0::2],
                                 start=True, stop=False)
                nc.tensor.matmul(ext[:], ebasis[:, 1:2], pusb[:, 1::2],
                                 start=False, stop=True)
                # de-interleave [ae | be | tt] into a staging tile, then one
                # contiguous 3 KB store into the DRAM scratch rows (scalar
                # HWDGE ring -- sync is busy streaming xT)
                svg = sa.tile([1, 3, 256], F32, tag="svg", name="svg")
                nc.vector.tensor_copy(
                    svg[:, 0:2, :],
                    pusb[0:1, :].rearrange("r (m v) -> r v m", v=2))
                nc.vector.tensor_copy(svg[:, 2, :], ext[:])
                nc.scalar.dma_start(vecd[:, g * 256:(g + 1) * 256], svg[:])

            def out_block(b):
                po = ps_a.tile([P, FOUT], F32, tag="po", name="po")
                xv = xtsl(b * P, (b + 1) * P)
                for c in range(4):
                    nc.tensor.matmul(po[:], xv[:, c, :], wbf[:, c, :],
                                     start=(c == 0), stop=(c == 3))
                h, kc = (0, b) if b < HKC else (1, b - HKC)
                if b % 2 == 0:
                    nc.scalar.copy(outb[h][:, kc, :FOUT], po[:])
                else:
                    nc.vector.tensor_copy(outb[h][:, kc, :FOUT], po[:])

            for s in range(8):
                u_group(2 * s)
                u_group(2 * s + 1)
                for b in range(8 * s, 8 * s + 8):
                    out_block(b)
                if s == 3:
                    # xT streaming is no longer the pacer; prefetch the
                    # first adj groups with the spare HBM bandwidth
                    adjg.extend(adj_load(g) for g in range(3))

            # ---- partition-major reload (SWDGE gathers) + activations ----
            for v in range(3):
                nc.gpsimd.dma_start(
                    rawpm[:, v * HKC:(v + 1) * HKC],
                    vecd[v:v + 1, :].rearrange("r (B p) -> (r p) B", p=P))
            lr = sa.tile([P, 3 * HKC], F32, tag="lr", name="lr")
            nc.vector.tensor_scalar_mul(lr[:], rawpm[:], 0.01)
            nc.vector.tensor_max(lr[:], rawpm[:], lr[:])
            nc.scalar.activation(expv[:], lr[:],
                                 mybir.ActivationFunctionType.Exp)
            # f = g + (1-g)*v  (per-partition scalars from gcol)
            nc.vector.tensor_scalar(fpm[:], expv[:, 2 * HKC:3 * HKC],
                                    gcol[:, 1:2], gcol[:, 0:1],
                                    op0=mybir.AluOpType.mult,
                                    op1=mybir.AluOpType.add)
            # alphas: al{1,2}[:, B'] = sum_B {ae,be}[:, B]*selg[:, B', B] + 1-g
            for bp in range(MB):
                m1 = sa.tile([P, HKC], F32, tag="alm", name="alm1")
                nc.vector.tensor_mul(m1[:], expv[:, 0:HKC], selg[:, bp, :])
                nc.vector.tensor_reduce(al1[:, bp:bp + 1], m1[:],
                                        axis=mybir.AxisListType.X,
                                        op=mybir.AluOpType.add)
                m2 = sa.tile([P, HKC], F32, tag="alm", name="alm2")
                nc.vector.tensor_mul(m2[:], expv[:, HKC:2 * HKC],
                                     selg[:, bp, :])
                nc.vector.tensor_reduce(al2[:, bp:bp + 1], m2[:],
                                        axis=mybir.AxisListType.X,
                                        op=mybir.AluOpType.add)
            nc.vector.tensor_scalar_add(al1[:], al1[:], gcol[:, 1:2])
            nc.vector.tensor_scalar_add(al2[:], al2[:], gcol[:, 1:2])

            # remaining adj loads (behind the gathers in the gpsimd FIFO;
            # slot waits resolve as stage B consumes groups)
            adjg.extend(adj_load(g) for g in range(3, NG))

            # rhs finalize: scale out rows by f, write f into column FOUT
            for h in range(2):
                nc.vector.tensor_copy(outb[h][:, :, FOUT:FOUT + 1], fpm[:])
            for b in range(NB):
                h, kc = (0, b) if b < HKC else (1, b - HKC)
                dst = outb[h][:, kc, :FOUT]
                if b % 2 == 0:
                    nc.vector.tensor_scalar_mul(dst, dst, fpm[:, kc:kc + 1])
                else:
                    nc.scalar.activation(dst, dst,
                                         mybir.ActivationFunctionType.Copy,
                                         scale=fpm[:, kc:kc + 1])

        # ---- stage B: Y = adjT.T @ outb, 8 PSUM banks per half ----
        with tc.tile_pool(name="ps_y", bufs=8, space="PSUM") as ps_y, \
             tc.tile_pool(name="comb", bufs=2) as comb:

            for h in range(2):
                yps = [ps_y.tile([P, FOUT + 1], F32, tag="yp",
                                 name=f"yp{h}_{m}") for m in range(MB)]
                for g in range(NG // 2):
                    at = adjg[h * (NG // 2) + g]
                    for t in range(GJ):
                        jc = g * GJ + t
                        for mb in range(MB):
                            nc.tensor.matmul(
                                yps[mb][:],
                                at[:, t, mb * P:(mb + 1) * P],
                                outb[h][:, jc, :],
                                start=(jc == 0), stop=(jc == HKC - 1))
                if h == 0:
                    for mb in range(MB):
                        nc.scalar.activation(
                            zsb[:, mb, :], yps[mb][:],
                            mybir.ActivationFunctionType.Copy,
                            scale=al1[:, mb:mb + 1])
                else:
                    for mb in range(MB):
                        t2 = comb.tile([P, FOUT + 1], F32, tag="t2",
                                       name="t2")
                        nc.vector.tensor_scalar_mul(t2[:], yps[mb][:],
                                                    al2[:, mb:mb + 1])
                        nc.vector.tensor_add(zsb[:, mb, :], zsb[:, mb, :],
                                             t2[:])

            # ---- combine + sigmoid + store ----
            for mb in range(MB):
                rec = comb.tile([P, 1], F32, tag="rec", name="rec")
                nc.vector.reciprocal(rec[:], zsb[:, mb, FOUT:FOUT + 1])
                res = comb.tile([P, FOUT], F32, tag="res", name="res")
                nc.vector.tensor_scalar_mul(res[:], zsb[:, mb, :FOUT], rec[:])
                resg = comb.tile([P, FOUT], F32, tag="resg", name="resg")
                nc.scalar.activation(resg[:], res[:],
                                     mybir.ActivationFunctionType.Sigmoid)
                nc.sync.dma_start(y_d.ap()[mb * P:(mb + 1) * P, :], resg[:])

    nc.compile()
    return nc


_NC_CACHE = None


def _get_program():
    global _NC_CACHE
    if _NC_CACHE is None:
        _NC_CACHE = build_program()
    return _NC_CACHE


def _bf16():
    import ml_dtypes
    return ml_dtypes.bfloat16


def make_in_maps(x, weight, att_vec, adj):
    bf16 = _bf16()
    x = np.asarray(x, dtype=np.float32)
    weight = np.ascontiguousarray(np.asarray(weight, dtype=np.float32))
    att_vec = np.asarray(att_vec, dtype=np.float32)
    adj8 = np.asarray(adj).astype(np.int8)

    xt = np.ascontiguousarray(x.T.astype(bf16))          # [FIN, N]
    attb = np.broadcast_to(att_vec[:, 0][None, :], (P, 2 * FOUT)).copy()
    ebasis = np.array([[0.0, 0.0], [1.0, 0.0], [0.0, 1.0]], np.float32)
    in_maps = []
    for c in range(NCORES):
        g = 1.0 if c < 4 else 0.0
        gcol = np.empty((P, 2), np.float32)
        gcol[:, 0] = g
        gcol[:, 1] = 1.0 - g
        selg = np.zeros((P, MB, HKC), np.float32)
        for bp in range(MB):
            selg[:, bp, (c * MB + bp) % HKC] = g
        adjt = np.ascontiguousarray(adj8[c * RPC:(c + 1) * RPC, :].T)
        in_maps.append({
            "xt": xt,
            "w": weight,
            "attb": attb,
            "adjt": adjt,
            "gcol": gcol,
            "selg": selg,
            "ebasis": ebasis,
        })
    return in_maps


def kernel(x, weight, att_vec, adj, _trace=False, _trace_kwargs=None):
    nc = _get_program()
    in_maps = make_in_maps(x, weight, att_vec, adj)
    r = run_bass_kernel_spmd(nc, in_maps, core_ids=list(range(NCORES)),
                             trace=_trace, **(_trace_kwargs or {}))
    y = np.concatenate([r.results[c]["y"] for c in range(NCORES)], axis=0)
    kernel.last_results = r
    return y.astype(np.float32)


# revision 16
# speedup vs baseline: 1.0578x; 1.0578x over previous
"""GAT layer (nn_GAT_layer_67619965108552) as a Trainium2 Bass/Tile SPMD kernel.

Structure exploited (same math as the verified baseline):
  With n=8192, the buggy-but-faithful pair indexing collapses:
    rows i < 4096:  scores[i, j] = u[2i + (j >= 4096)],  u = x @ (W@a1 + W@a2)
    rows i >= 4096: scores[i, j] = tt[j mod 4096],       tt = s1[even] + s2[odd]
  After leaky_relu + adj masking + softmax, attn @ out reduces to two masked
  row-sum matmuls against adj halves:
    Y1 = A[:, :4096] @ [f*out_L | f],  Y2 = A[:, 4096:] @ [f*out_R | f]
    res = sigmoid((al1*Y1 + al2*Y2)[:, :256] / (al1*Y1 + al2*Y2)[:, 256])
  Top-half cores: f = 1, al1 = exp(lrelu(u_even)), al2 = exp(lrelu(u_odd));
  bottom-half cores: f = exp(lrelu(tt)), al1 = al2 = 1. Same instruction
  stream on all cores; only input data (g / select masks) differs.

Layout strategy (this is where the speed comes from vs the old version):
  * x arrives host-pre-transposed as bf16 xT [512, 8192] -> matmul lhsT/rhs
    tiles load straight from DRAM; zero on-chip transposes.
  * each core's adj slice arrives host-pre-transposed as int8 adjT [8192,1024]
    -> SWDGE dma casts i8->bf16 on load; tiles are directly the lhsT of the
    Y matmuls; zero on-chip transposes and 4x less HBM traffic.
  * U-pass (scores) runs before the out-pass; score vectors scatter to a DRAM
    scratch DURING the U-pass and reload once partition-major, so leaky_relu/
    exp run on [128, 96] tiles and the tensor engine never idles waiting.
  * Stage B accumulates Y in 8 PSUM banks per half directly.

Sharding: rows of adj (and of the output) across 8 cores, 1024 rows each.
x/weight/att_vec replicated; every core computes the full out = x@W.
"""
import numpy as np
from contextlib import ExitStack

import concourse.bass as bass
import concourse.tile as tile
from concourse import bacc, mybir
from concourse.bass_utils import run_bass_kernel_spmd

F32 = mybir.dt.float32
BF16 = mybir.dt.bfloat16
I8 = mybir.dt.int8

N = 8192          # nodes
FIN = 512         # input features
FOUT = 256        # output features
P = 128
NB = N // P       # 64 j-chunks over all nodes
NCORES = 8
RPC = N // NCORES  # 1024 rows per core
MB = RPC // P      # 8 output row-blocks per core
HKC = 32           # j-chunks per half (4096/128)
GJ = 8             # j-chunks per adj DMA group
NG = NB // GJ      # 8 adj groups
UG = 16            # U-pass groups of 512 nodes


def build_program():
    nc = bacc.Bacc("TRN2", target_bir_lowering=False, debug=False,
                   num_devices=NCORES)

    xt_d = nc.dram_tensor("xt", [FIN, N], BF16, kind="ExternalInput")
    w_d = nc.dram_tensor("w", [FIN, FOUT], F32, kind="ExternalInput")
    attb_d = nc.dram_tensor("attb", [P, 2 * FOUT], F32, kind="ExternalInput")
    adjt_d = nc.dram_tensor("adjt", [N, RPC], I8, kind="ExternalInput")
    # gcol[:, 0] = g (1.0 for top-half cores, 0.0 for bottom), gcol[:, 1] = 1-g
    g_d = nc.dram_tensor("gcol", [P, 2], F32, kind="ExternalInput")
    # selg[p, B', B] = g * (B == 8c + B') : per-core row-block select
    selg_d = nc.dram_tensor("selg", [P, MB, HKC], F32, kind="ExternalInput")
    # basis vectors for psum row extraction: col 0 -> row 1, col 1 -> row 2
    eb_d = nc.dram_tensor("ebasis", [3, 2], F32, kind="ExternalInput")
    y_d = nc.dram_tensor("y", [RPC, FOUT], F32, kind="ExternalOutput")

    with tile.TileContext(nc) as tc, ExitStack() as ctx:
        constp = ctx.enter_context(tc.tile_pool(name="const", bufs=1))
        dramp = ctx.enter_context(tc.tile_pool(name="dram", bufs=1, space="DRAM"))
        # adj tiles stream through 3 slots; DMAs issued up-front so they run
        # under stage A (gpsimd/SWDGE queue carries nothing else).
        adjp = ctx.enter_context(tc.tile_pool(name="adjp", bufs=3))

        # ---- constants ----
        wtile = constp.tile([P, 4, FOUT], F32)     # W, k-chunk major
        nc.sync.dma_start(wtile[:], w_d.ap().rearrange("(c p) f -> p c f", p=P))
        wbf = constp.tile([P, 4, FOUT], BF16)
        nc.vector.tensor_copy(wbf[:], wtile[:])
        attb = constp.tile([P, 2 * FOUT], F32)     # [a1 | a2], partition-bcast
        nc.sync.dma_start(attb[:], attb_d.ap())
        gcol = constp.tile([P, 2], F32)
        nc.sync.dma_start(gcol[:], g_d.ap())
        selg = constp.tile([P, MB, HKC], F32)
        nc.sync.dma_start(selg[:], selg_d.ap())
        ebasis = constp.tile([3, 2], F32)
        nc.sync.dma_start(ebasis[:], eb_d.ap())

        # persistent mid-size tensors
        outb = [constp.tile([P, HKC, FOUT + 1], BF16, name=f"outb{h}")
                for h in range(2)]
        zsb = constp.tile([P, MB, FOUT + 1], F32)
        rawpm = constp.tile([P, 3 * HKC], F32)   # [ae_raw | be_raw | tt_raw]
        expv = constp.tile([P, 3 * HKC], F32)    # exp(lrelu(rawpm))
        fpm = constp.tile([P, HKC], F32)
        al1 = constp.tile([P, MB], F32)
        al2 = constp.tile([P, MB], F32)
        wamf = constp.tile([P, 4, 3], F32)
        wam = constp.tile([P, 4, 3], BF16)

        # DRAM scratch for the free-major -> partition-major shuffle of the
        # score vectors: rows = [ae | be | tt], each 4096 long, index-major.
        vecd = dramp.tile([3, N // 2], F32)

        def adj_load(g):
            t = adjp.tile([P, GJ, RPC], BF16, tag="adjg", name=f"adjg{g}")
            nc.gpsimd.dma_start(
                t[:],
                adjt_d.ap()[g * GJ * P:(g + 1) * GJ * P, :].rearrange(
                    "(t p) i -> p t i", p=P))
            return t

        # adj loads are all deferred until after the stage-A gather DMAs:
        # they'd compete with the xT stream for HBM (the U-pass is paced by
        # it) and anything slot-waiting on stage-B consumption must not sit
        # ahead of the gathers in the gpsimd FIFO.
        adjg = []

        # ---- stage A (scoped pools) ----
        with tc.tile_pool(name="xtp", bufs=1) as xtp, \
             tc.tile_pool(name="sa", bufs=3) as sa, \
             tc.tile_pool(name="ps_u", bufs=2, space="PSUM") as ps_u, \
             tc.tile_pool(name="ps_e", bufs=2, space="PSUM") as ps_e, \
             tc.tile_pool(name="ps_a", bufs=4, space="PSUM") as ps_a:

            # resident xT [k-part, c, n] as 8 slice tiles of 1 MB each, so
            # the U-pass only waits on the slices it has reached
            NS = N // 8
            xts = []
            for s in range(8):
                sl = slice(s * NS, (s + 1) * NS)
                xs = xtp.tile([P, 4, NS], BF16, name=f"xt{s}")
                nc.sync.dma_start(
                    xs[:],
                    xt_d.ap()[:, sl].rearrange("(c p) n -> p c n", p=P))
                xts.append(xs)

            def xtsl(n0, n1):
                """[P, 4, n1-n0] view of the xT slice tile covering n0:n1."""
                s = n0 // NS
                assert (n1 - 1) // NS == s
                return xts[s][:, :, n0 - s * NS:n1 - s * NS]

            # wam[:, c, :] = [wu | wa1 | wa2] chunk c (bf16 lhsT for U-matmul)
            for c in range(4):
                t = sa.tile([P, FOUT], F32, tag="wa_tmp", name="wa_tmp")
                nc.vector.tensor_mul(t[:], wtile[:, c, :], attb[:, :FOUT])
                nc.vector.tensor_reduce(wamf[:, c, 1:2], t[:],
                                        axis=mybir.AxisListType.X,
                                        op=mybir.AluOpType.add)
                t2 = sa.tile([P, FOUT], F32, tag="wa_tmp", name="wa_tmp2")
                nc.vector.tensor_mul(t2[:], wtile[:, c, :], attb[:, FOUT:])
                nc.vector.tensor_reduce(wamf[:, c, 2:3], t2[:],
                                        axis=mybir.AxisListType.X,
                                        op=mybir.AluOpType.add)
                nc.vector.tensor_add(wamf[:, c, 0:1], wamf[:, c, 1:2],
                                     wamf[:, c, 2:3])
            nc.vector.tensor_copy(wam[:], wamf[:])

            # ---- U pass, ext extraction software-pipelined one group
            # behind so the in-order PE queue never waits on the DVE
            # PSUM->SBUF copy (which would gap the PE and keep HAM cold).
            def u_mms(g):
                pu = ps_u.tile([3, 512], F32, tag="pu", name="pu")
                for c in range(4):
                    nc.tensor.matmul(pu[:], wam[:, c, :],
                                     xtsl(g * 512, (g + 1) * 512)[:, c, :],
                                     start=(c == 0), stop=(c == 3))
                pusb = sa.tile([3, 512], F32, tag="pusb", name="pusb")
                if g % 2 == 0:
                    nc.vector.tensor_copy(pusb[:], pu[:])
                else:
                    nc.scalar.copy(pusb[:], pu[:])
                return pusb

            def u_ext(g, pusb):
                # tt = s1[even] + s2[odd]: two basis extractions into one PSUM
                ext = ps_e.tile([1, 256], F32, tag="ext", name="ext")
                nc.tensor.matmul(ext[:], ebasis[:, 0:1], pusb[:, 0::2],
                                 start=True, stop=False)
                nc.tensor.matmul(ext[:], ebasis[:, 1:2], pusb[:, 1::2],
                                 start=False, stop=True)
                # de-interleave [ae | be | tt] into a staging tile, then one
                # contiguous 3 KB store into the DRAM scratch rows (scalar
                # HWDGE ring -- sync is busy streaming xT)
                svg = sa.tile([1, 3, 256], F32, tag="svg", name="svg")
                nc.vector.tensor_copy(
                    svg[:, 0:2, :],
                    pusb[0:1, :].rearrange("r (m v) -> r v m", v=2))
                nc.vector.tensor_copy(svg[:, 2, :], ext[:])
                nc.scalar.dma_start(vecd[:, g * 256:(g + 1) * 256], svg[:])

            pend = None
            for g in range(UG):
                pusb = u_mms(g)
                if pend is not None:
                    u_ext(*pend)
                pend = (g, pusb)
            u_ext(*pend)

            # ---- partition-major reload (SWDGE gathers) + activations ----
            for v in range(3):
                nc.gpsimd.dma_start(
                    rawpm[:, v * HKC:(v + 1) * HKC],
                    vecd[v:v + 1, :].rearrange("r (B p) -> (r p) B", p=P))
            lr = sa.tile([P, 3 * HKC], F32, tag="lr", name="lr")
            nc.vector.tensor_scalar_mul(lr[:], rawpm[:], 0.01)
            nc.vector.tensor_max(lr[:], rawpm[:], lr[:])
            nc.scalar.activation(expv[:], lr[:],
                                 mybir.ActivationFunctionType.Exp)
            # f = g + (1-g)*v  (per-partition scalars from gcol)
            nc.vector.tensor_scalar(fpm[:], expv[:, 2 * HKC:3 * HKC],
                                    gcol[:, 1:2], gcol[:, 0:1],
                                    op0=mybir.AluOpType.mult,
                                    op1=mybir.AluOpType.add)
            # alphas: al{1,2}[:, B'] = sum_B {ae,be}[:, B]*selg[:, B', B] + 1-g
            for bp in range(MB):
                m1 = sa.tile([P, HKC], F32, tag="alm", name="alm1")
                nc.vector.tensor_mul(m1[:], expv[:, 0:HKC], selg[:, bp, :])
                nc.vector.tensor_reduce(al1[:, bp:bp + 1], m1[:],
                                        axis=mybir.AxisListType.X,
                                        op=mybir.AluOpType.add)
                m2 = sa.tile([P, HKC], F32, tag="alm", name="alm2")
                nc.vector.tensor_mul(m2[:], expv[:, HKC:2 * HKC],
                                     selg[:, bp, :])
                nc.vector.tensor_reduce(al2[:, bp:bp + 1], m2[:],
                                        axis=mybir.AxisListType.X,
                                        op=mybir.AluOpType.add)
            nc.vector.tensor_scalar_add(al1[:], al1[:], gcol[:, 1:2])
            nc.vector.tensor_scalar_add(al2[:], al2[:], gcol[:, 1:2])

            # adj loads (behind the gathers in the gpsimd FIFO; slot waits
            # of groups 3+ resolve as stage B consumes groups)
            adjg.extend(adj_load(g) for g in range(NG))

            # ---- out pass: out = x @ W, blocks land in outb ----
            for b in range(NB):
                po = ps_a.tile([P, FOUT], F32, tag="po", name="po")
                xv = xtsl(b * P, (b + 1) * P)
                for c in range(4):
                    nc.tensor.matmul(po[:], xv[:, c, :], wbf[:, c, :],
                                     start=(c == 0), stop=(c == 3))
                h, kc = (0, b) if b < HKC else (1, b - HKC)
                if b % 2 == 0:
                    nc.scalar.copy(outb[h][:, kc, :FOUT], po[:])
                else:
                    nc.vector.tensor_copy(outb[h][:, kc, :FOUT], po[:])

            # rhs finalize: scale out rows by f, write f into column FOUT
            for h in range(2):
                nc.vector.tensor_copy(outb[h][:, :, FOUT:FOUT + 1], fpm[:])
            for b in range(NB):
                h, kc = (0, b) if b < HKC else (1, b - HKC)
                dst = outb[h][:, kc, :FOUT]
                if b % 2 == 0:
                    nc.vector.tensor_scalar_mul(dst, dst, fpm[:, kc:kc + 1])
                else:
                    nc.scalar.activation(dst, dst,
                                         mybir.ActivationFunctionType.Copy,
                                         scale=fpm[:, kc:kc + 1])

        # ---- stage B: Y = adjT.T @ outb, 8 PSUM banks per half ----
        with tc.tile_pool(name="ps_y", bufs=8, space="PSUM") as ps_y, \
             tc.tile_pool(name="comb", bufs=2) as comb:

            for h in range(2):
                yps = [ps_y.tile([P, FOUT + 1], F32, tag="yp",
                                 name=f"yp{h}_{m}") for m in range(MB)]
                for g in range(NG // 2):
                    at = adjg[h * (NG // 2) + g]
                    for t in range(GJ):
                        jc = g * GJ + t
                        for mb in range(MB):
                            nc.tensor.matmul(
                                yps[mb][:],
                                at[:, t, mb * P:(mb + 1) * P],
                                outb[h][:, jc, :],
                                start=(jc == 0), stop=(jc == HKC - 1))
                if h == 0:
                    for mb in range(MB):
                        nc.scalar.activation(
                            zsb[:, mb, :], yps[mb][:],
                            mybir.ActivationFunctionType.Copy,
                            scale=al1[:, mb:mb + 1])
                else:
                    for mb in range(MB):
                        t2 = comb.tile([P, FOUT + 1], F32, tag="t2",
                                       name="t2")
                        nc.vector.tensor_scalar_mul(t2[:], yps[mb][:],
                                                    al2[:, mb:mb + 1])
                        nc.vector.tensor_add(zsb[:, mb, :], zsb[:, mb, :],
                                             t2[:])

            # ---- combine + sigmoid + store ----
            for mb in range(MB):
                rec = comb.tile([P, 1], F32, tag="rec", name="rec")
                nc.vector.reciprocal(rec[:], zsb[:, mb, FOUT:FOUT + 1])
                res = comb.tile([P, FOUT], F32, tag="res", name="res")
                nc.vector.tensor_scalar_mul(res[:], zsb[:, mb, :FOUT], rec[:])
                resg = comb.tile([P, FOUT], F32, tag="resg", name="resg")
                nc.scalar.activation(resg[:], res[:],
                                     mybir.ActivationFunctionType.Sigmoid)
                nc.sync.dma_start(y_d.ap()[mb * P:(mb + 1) * P, :], resg[:])

    nc.compile()
    return nc


_NC_CACHE = None


def _get_program():
    global _NC_CACHE
    if _NC_CACHE is None:
        _NC_CACHE = build_program()
    return _NC_CACHE


def _bf16():
    import ml_dtypes
    return ml_dtypes.bfloat16


def make_in_maps(x, weight, att_vec, adj):
    bf16 = _bf16()
    x = np.asarray(x, dtype=np.float32)
    weight = np.ascontiguousarray(np.asarray(weight, dtype=np.float32))
    att_vec = np.asarray(att_vec, dtype=np.float32)
    adj8 = np.asarray(adj).astype(np.int8)

    xt = np.ascontiguousarray(x.T.astype(bf16))          # [FIN, N]
    attb = np.broadcast_to(att_vec[:, 0][None, :], (P, 2 * FOUT)).copy()
    ebasis = np.array([[0.0, 0.0], [1.0, 0.0], [0.0, 1.0]], np.float32)
    in_maps = []
    for c in range(NCORES):
        g = 1.0 if c < 4 else 0.0
        gcol = np.empty((P, 2), np.float32)
        gcol[:, 0] = g
        gcol[:, 1] = 1.0 - g
        selg = np.zeros((P, MB, HKC), np.float32)
        for bp in range(MB):
            selg[:, bp, (c * MB + bp) % HKC] = g
        adjt = np.ascontiguousarray(adj8[c * RPC:(c + 1) * RPC, :].T)
        in_maps.append({
            "xt": xt,
            "w": weight,
            "attb": attb,
            "adjt": adjt,
            "gcol": gcol,
            "selg": selg,
            "ebasis": ebasis,
        })
    return in_maps


def kernel(x, weight, att_vec, adj, _trace=False, _trace_kwargs=None):
    nc = _get_program()
    in_maps = make_in_maps(x, weight, att_vec, adj)
    r = run_bass_kernel_spmd(nc, in_maps, core_ids=list(range(NCORES)),
                             trace=_trace, **(_trace_kwargs or {}))
    y = np.concatenate([r.results[c]["y"] for c in range(NCORES)], axis=0)
    kernel.last_results = r
    return y.astype(np.float32)
